# revision 3
# baseline (speedup 1.0000x reference)
"""Trainium2 Bass kernel for the NTN problem.

out[b,k,q,a] = sigmoid( q[b,q,:] @ w[k] @ da[b,a,:]
                        + Vq[k]@q[b,q,:] + Vd[k]@da[b,a,:] + b[k] )

B=64, K=16, Q=A=D=256.  Sharding: data-parallel over batch B across the
8 NeuronCores (8 batches per core); w/V/b replicated.

Per core, per (k, batch-pair):
  MM1 (TensorE, fp16): tmp[e, q|q'] = sum_d w[k,d,e]^T qT[d, q|q']   (N=512)
  DVE: tmp PSUM->SBUF (fp16) with per-partition bias +Vd[k,e] (folds Vd@da)
  MM2 (TensorE, fp16): out[q, a] = sum_e tmp[e,q]^T daT[e, a]
  ScalarE: sigmoid(psum + bias mq[b,k,q]) where mq = Vq@q + b (host-prepped),
  written as fp16 into per-(b, qtile, k-chunk) collect tiles; the host
  upcasts the returned fp16 output to fp32 (quantization err ~2.4e-4,
  well inside the 2e-2 budget) which halves store traffic 32->16 MB/core.

Startup path (from NTFF traces): engine preambles run until ~5us, first
DMA descriptors flow ~8.7us.  The critical first loads are split across
the two HWDGE queues (scalar: q2 pair0; SP: w[0] first), with the rest of
the SP queue in consumer-deadline order (w1, da2, vdt, mq, w2..).  PE
warm-up matmuls on an (uninitialized) scratch tile run from the moment
the PE sequencer clears its preamble, so the HAM clock is at full speed
when the real stream starts; no memset gates them (the old GpSimd memset
pushed warm-up start to 7.7us).
"""

import os
import sys
import types
from contextlib import ExitStack

if "/opt/trn_rl_repo" not in sys.path:
    sys.path.insert(0, "/opt/trn_rl_repo")

import numpy as np

import concourse.bass as bass
import concourse.tile as tile
from concourse import bacc, bass_utils, mybir

F32 = mybir.dt.float32
F16 = mybir.dt.float16
SIG = mybir.ActivationFunctionType.Sigmoid

NCORES = 8
B, Q, A, D, K = 64, 256, 256, 256, 16
E = D
BL = B // NCORES

N_WARMUP = 13


def _install_profshim():
    """Provide antenv.axon_hooks so trace=True works under axon (best-effort)."""
    try:
        if "antenv.axon_hooks" in sys.modules:
            return True
        import antenv

        mod = types.ModuleType("antenv.axon_hooks")
        holder = {}
        mod.set_axon_ntff_profile_hook = lambda h: holder.__setitem__("h", h)
        mod.get_axon_ntff_profile_hook = lambda: holder.get("h")
        sys.modules["antenv.axon_hooks"] = mod
        antenv.axon_hooks = mod
        from trn_agent_boot.trn_boot import _ntff_profile_via_ctypes

        hook = _ntff_profile_via_ctypes("/opt/axon/libaxon_pjrt.so")
        if hook is None:
            return False
        mod.set_axon_ntff_profile_hook(hook)
        return True
    except Exception:
        return False


def _build_ntn(tc: tile.TileContext, ctx: ExitStack, aps: dict):
    nc = tc.nc
    DC, ET, QT = D // 128, E // 128, Q // 128
    qt, dat, w, vdt, mq, out = (aps[n] for n in ("qt", "dat", "w", "vdt", "mq", "out"))

    w_pool = ctx.enter_context(tc.tile_pool(name="w", bufs=1))
    const_pool = ctx.enter_context(tc.tile_pool(name="const", bufs=1))
    q_pool = ctx.enter_context(tc.tile_pool(name="q", bufs=3))
    da_pool = ctx.enter_context(tc.tile_pool(name="da", bufs=3))
    tmp_pool = ctx.enter_context(tc.tile_pool(name="tmp", bufs=4))
    out_pool = ctx.enter_context(tc.tile_pool(name="out", bufs=24))
    ptmp_pool = ctx.enter_context(tc.tile_pool(name="ptmp", bufs=3, space="PSUM"))
    pout_pool = ctx.enter_context(tc.tile_pool(name="pout", bufs=5, space="PSUM"))

    act_tiles = {}

    # PE warm-up: dummy matmuls on a scratch tile keep the PE busy from the
    # end of its sequencer preamble until the first real loads land, so HAM
    # reaches full clock before the real stream starts.  The memset rides on
    # DVE (idle until the first k's bias-add, and its preamble clears early;
    # the old GpSimd memset pushed warm-up start to 7.7us).
    warm_pool = ctx.enter_context(tc.tile_pool(name="warm", bufs=1))
    scratch = warm_pool.tile([128, 512], F16)
    nc.vector.memset(scratch[:], 0.0)
    pwarm = ptmp_pool.tile([128, 512], F32, name="pwarm", tag="pt")
    for _ in range(N_WARMUP):
        nc.tensor.matmul(pwarm[:], lhsT=scratch[:, 0:128], rhs=scratch[:],
                         start=True, stop=True)

    def load_pair(bp, first=False):
        b0, b1 = 2 * bp, 2 * bp + 1
        q2 = q_pool.tile([128, DC, 2 * Q], F16, name=f"q2_{bp}", tag="q2")
        nc.sync.dma_start(q2[:].rearrange("p dc (h q) -> p dc h q", h=2),
                          qt[b0:b0 + 2].rearrange("h p dc q -> p dc h q"))
        if first:
            return (q2, b0, b1)
        da2 = da_pool.tile([128, ET, 2 * A], F16, name=f"da2_{bp}", tag="da2")
        nc.sync.dma_start(da2[:].rearrange("p et (h a) -> p et h a", h=2),
                          dat[b0:b0 + 2].rearrange("h p et a -> p et h a"))
        act_tiles[bp] = (q2, da2)

    def load_wk(k):
        wk = w_pool.tile([128, DC, E], F16, name=f"wk{k}", tag=f"wk{k}")
        nc.sync.dma_start(wk[:], w[k].rearrange("(dc p) e -> p dc e", p=128))
        return wk

    # Critical path: MM1(k0) needs q2_0 + wk0.  Put q2_0 (256KB) alone on the
    # scalar HWDGE queue and wk0 (128KB) first on the SP queue so both stream
    # concurrently the moment descriptors start flowing; the rest of SP's
    # queue follows consumer deadlines: MM1(k1)->MM2(k0)->DVE(k0)->ACT(k0).
    q2_0 = q_pool.tile([128, DC, 2 * Q], F16, name="q2_0", tag="q2")
    nc.scalar.dma_start(q2_0[:].rearrange("p dc (h q) -> p dc h q", h=2),
                        qt[0:2].rearrange("h p dc q -> p dc h q"))
    w_sb = {}
    w_sb[0] = load_wk(0)
    w_sb[1] = load_wk(1)
    da2_0 = da_pool.tile([128, ET, 2 * A], F16, name="da2_0", tag="da2")
    nc.sync.dma_start(da2_0[:].rearrange("p et (h a) -> p et h a", h=2),
                      dat[0:2].rearrange("h p et a -> p et h a"))
    act_tiles[0] = (q2_0, da2_0)
    # small constants ride on the (otherwise idle) scalar queue after q2_0
    vdt_sb = const_pool.tile([128, ET, 128], F32)
    nc.scalar.dma_start(vdt_sb[:], vdt.rearrange("et p k -> p et k"))
    mq_sb = const_pool.tile([128, QT, BL, K], F32)
    nc.scalar.dma_start(mq_sb[:], mq.rearrange("t p b k -> p t b k"))
    for k in range(2, K):
        w_sb[k] = load_wk(k)

    NBP = BL // 2
    for bp in range(NBP):
        b0, b1 = 2 * bp, 2 * bp + 1
        if bp not in act_tiles:
            load_pair(bp)
        if bp + 1 < NBP and bp + 1 not in act_tiles:
            load_pair(bp + 1)
        q2, da2 = act_tiles.pop(bp)

        # per-(b, qtile, k-chunk) collect tiles (fp16); chunks shrink toward
        # the end of the last batch-pair so the final store flush is small
        if bp < NBP - 1:
            chunk_sizes = [8, 8]
        else:
            chunk_sizes = [4, 4, 4, 2, 1, 1]
        k2chunk = {}
        koff = 0
        for ci, cs in enumerate(chunk_sizes):
            for off in range(cs):
                k2chunk[koff + off] = (ci, off, cs)
            koff += cs
        coll = {(h, qt_i, ci): out_pool.tile([128, cs, A], F16, name="coll", tag="coll")
                for h in (0, 1) for qt_i in range(QT)
                for ci, cs in enumerate(chunk_sizes)}

        for k in range(K):
            ptmps = []
            for et in range(ET):
                pt = ptmp_pool.tile([128, 2 * Q], F32)
                for dc in range(DC):
                    nc.tensor.matmul(
                        pt[:],
                        lhsT=w_sb[k][:, dc, et * 128:(et + 1) * 128],
                        rhs=q2[:, dc, :],
                        start=(dc == 0),
                        stop=(dc == DC - 1),
                    )
                ptmps.append(pt)
            tmp = tmp_pool.tile([128, ET, 2 * Q], F16)
            for et in range(ET):
                nc.vector.tensor_scalar_add(
                    tmp[:, et, :], ptmps[et][:], vdt_sb[:, et, k:k + 1]
                )
            for h, b in ((0, b0), (1, b1)):
                for qt_i in range(QT):
                    po = pout_pool.tile([128, A], F32)
                    for et in range(ET):
                        nc.tensor.matmul(
                            po[:],
                            lhsT=tmp[:, et, h * Q + qt_i * 128: h * Q + (qt_i + 1) * 128],
                            rhs=da2[:, et, h * A:(h + 1) * A],
                            start=(et == 0),
                            stop=(et == ET - 1),
                        )
                    nc.scalar.activation(
                        coll[(h, qt_i, k2chunk[k][0])][:, k2chunk[k][1], :], po[:], SIG,
                        bias=mq_sb[:, qt_i, b, k:k + 1],
                    )
            ci, off, cs = k2chunk[k]
            if off == cs - 1:
                k_lo = k - cs + 1
                for h, b in ((0, b0), (1, b1)):
                    for qt_i in range(QT):
                        # split issue across SP (h=0) and ScalarE (h=1, whose
                        # queue is idle after startup) to halve serialization
                        eng = nc.scalar if h == 1 else nc.sync
                        eng.dma_start(
                            out[b, k_lo:k_lo + cs,
                                qt_i * 128:(qt_i + 1) * 128, :].rearrange("k p a -> p k a"),
                            coll[(h, qt_i, ci)][:],
                        )


_COMPILED = None


def _get_compiled():
    global _COMPILED
    if _COMPILED is not None:
        return _COMPILED
    nc = bacc.Bacc("TRN2", target_bir_lowering=False, debug=False, num_devices=NCORES)
    aps = {
        "qt": nc.dram_tensor("qt", [BL, 128, D // 128, Q], F16, kind="ExternalInput").ap(),
        "dat": nc.dram_tensor("dat", [BL, 128, E // 128, A], F16, kind="ExternalInput").ap(),
        "w": nc.dram_tensor("w", [K, D, E], F16, kind="ExternalInput").ap(),
        "vdt": nc.dram_tensor("vdt", [E // 128, 128, 128], F32, kind="ExternalInput").ap(),
        "mq": nc.dram_tensor("mq", [Q // 128, 128, BL, K], F32, kind="ExternalInput").ap(),
        "out": nc.dram_tensor("out", [BL, K, Q, A], F16, kind="ExternalOutput").ap(),
    }
    with tile.TileContext(nc) as tc:
        with ExitStack() as ctx:
            _build_ntn(tc, ctx, aps)
    nc.compile()
    _COMPILED = nc
    return nc


def kernel(batch_q_em, batch_da_em, w, V, b):
    q = np.ascontiguousarray(np.asarray(batch_q_em, dtype=np.float32))
    da = np.ascontiguousarray(np.asarray(batch_da_em, dtype=np.float32))
    w = np.ascontiguousarray(np.asarray(w, dtype=np.float32))
    V = np.ascontiguousarray(np.asarray(V, dtype=np.float32))
    b = np.asarray(b, dtype=np.float32).reshape(-1)

    # packed to SBUF layout [b, p, dc, q] so each load is 128 x 1KB descriptors
    qt = np.ascontiguousarray(
        q.transpose(0, 2, 1).reshape(B, D // 128, 128, Q).transpose(0, 2, 1, 3)
    ).astype(np.float16)                                  # [B, 128, DC, Q]
    dat = np.ascontiguousarray(
        da.transpose(0, 2, 1).reshape(B, E // 128, 128, A).transpose(0, 2, 1, 3)
    ).astype(np.float16)                                  # [B, 128, ET, A]
    w16 = w.astype(np.float16)
    vdt_cols = np.ascontiguousarray(V[:, D:].T)          # [E, K]
    vdt = np.zeros((E // 128, 128, 128), dtype=np.float32)
    vdt[:, :, :K] = vdt_cols.reshape(E // 128, 128, K)
    # mq[b,q,k] = q[b] @ Vq^T + bias
    mqT = q @ V[:, :D].T + b[None, None, :]              # [B, Q, K]

    nc = _get_compiled()
    in_maps = []
    for c in range(NCORES):
        s = slice(c * BL, (c + 1) * BL)
        mq_shard = np.ascontiguousarray(
            mqT[s].reshape(BL, Q // 128, 128, K).transpose(1, 2, 0, 3)
        )  # [QT, 128, BL, K]
        in_maps.append({
            "qt": np.ascontiguousarray(qt[s]),
            "dat": np.ascontiguousarray(dat[s]),
            "w": w16,
            "vdt": vdt,
            "mq": mq_shard,
        })

    trace = bool(int(os.environ.get("NTN_TRACE", "0"))) and _install_profshim()
    res = bass_utils.run_bass_kernel_spmd(
        nc, in_maps, core_ids=list(range(NCORES)), trace=trace
    )
    if trace and res.exec_time_ns is not None:
        print(f"HW exec time: {res.exec_time_ns} ns")
    out = np.concatenate([r["out"] for r in res.results], axis=0).astype(np.float32)
    return out


# revision 10
# speedup vs baseline: 1.0717x; 1.0717x over previous
"""Trainium2 Bass kernel for the NTN problem.

out[b,k,q,a] = sigmoid( q[b,q,:] @ w[k] @ da[b,a,:]
                        + Vq[k]@q[b,q,:] + Vd[k]@da[b,a,:] + b[k] )

B=64, K=16, Q=A=D=256.  Sharding: data-parallel over batch B across the
8 NeuronCores (8 batches per core); w/V/b replicated.

Per core, per (k, batch-pair):
  MM1 (TensorE, fp16): tmp[e, q|q'] = sum_d w[k,d,e]^T qT[d, q|q']   (N=512)
  DVE: tmp PSUM->SBUF (fp16) with per-partition bias +Vd[k,e] (folds Vd@da)
  MM2 (TensorE, fp16): out[q, a] = sum_e tmp[e,q]^T daT[e, a]
  ScalarE: sigmoid(psum + bias mq[b,k,q]) where mq = Vq@q + b (host-prepped),
  written as fp16 into per-(b, qtile, k-chunk) collect tiles; the host
  upcasts the returned fp16 output to fp32 (quantization err ~2.4e-4,
  well inside the 2e-2 budget) which halves store traffic 32->16 MB/core.

Startup path (from NTFF traces): engine preambles run until ~5us, first
DMA descriptors flow ~8.7us.  The critical first loads are split across
the two HWDGE queues (scalar: q2 pair0; SP: w[0] first), with the rest of
the SP queue in consumer-deadline order (w1, da2, vdt, mq, w2..).  PE
warm-up matmuls on an (uninitialized) scratch tile run from the moment
the PE sequencer clears its preamble, so the HAM clock is at full speed
when the real stream starts; no memset gates them (the old GpSimd memset
pushed warm-up start to 7.7us).
"""

import os
import sys
import types
from contextlib import ExitStack

if "/opt/trn_rl_repo" not in sys.path:
    sys.path.insert(0, "/opt/trn_rl_repo")

import numpy as np

import concourse.bass as bass
import concourse.tile as tile
from concourse import bacc, bass_utils, mybir

F32 = mybir.dt.float32
F16 = mybir.dt.float16
SIG = mybir.ActivationFunctionType.Sigmoid

NCORES = 8
B, Q, A, D, K = 64, 256, 256, 256, 16
E = D
BL = B // NCORES

N_WARMUP = 16


def _install_profshim():
    """Provide antenv.axon_hooks so trace=True works under axon (best-effort)."""
    try:
        if "antenv.axon_hooks" in sys.modules:
            return True
        import antenv

        mod = types.ModuleType("antenv.axon_hooks")
        holder = {}
        mod.set_axon_ntff_profile_hook = lambda h: holder.__setitem__("h", h)
        mod.get_axon_ntff_profile_hook = lambda: holder.get("h")
        sys.modules["antenv.axon_hooks"] = mod
        antenv.axon_hooks = mod
        from trn_agent_boot.trn_boot import _ntff_profile_via_ctypes

        hook = _ntff_profile_via_ctypes("/opt/axon/libaxon_pjrt.so")
        if hook is None:
            return False
        mod.set_axon_ntff_profile_hook(hook)
        return True
    except Exception:
        return False


def _build_ntn(tc: tile.TileContext, ctx: ExitStack, aps: dict):
    nc = tc.nc
    DC, ET, QT = D // 128, E // 128, Q // 128
    qt, dat, w, vdt, mq, out = (aps[n] for n in ("qt", "dat", "w", "vdt", "mq", "out"))

    w_pool = ctx.enter_context(tc.tile_pool(name="w", bufs=1))
    const_pool = ctx.enter_context(tc.tile_pool(name="const", bufs=1))
    q_pool = ctx.enter_context(tc.tile_pool(name="q", bufs=3))
    da_pool = ctx.enter_context(tc.tile_pool(name="da", bufs=3))
    tmp_pool = ctx.enter_context(tc.tile_pool(name="tmp", bufs=4))
    out_pool = ctx.enter_context(tc.tile_pool(name="out", bufs=24))
    ptmp_pool = ctx.enter_context(tc.tile_pool(name="ptmp", bufs=3, space="PSUM"))
    pout_pool = ctx.enter_context(tc.tile_pool(name="pout", bufs=5, space="PSUM"))

    act_tiles = {}

    # PE warm-up: dummy matmuls on a scratch tile keep the PE busy from the
    # end of its sequencer preamble until the first real loads land, so HAM
    # reaches full clock before the real stream starts.  The memset rides on
    # DVE (idle until the first k's bias-add, and its preamble clears early;
    # the old GpSimd memset pushed warm-up start to 7.7us).  256-row warm
    # matmuls give fine granularity, so the warm-up end lands close to the
    # moment the first loads are ready.  pwarm shares ptmp's ring via the
    # explicit "pt" tag: a pool splits bufs across tags, so an untagged
    # pwarm would silently shrink the per-k pt ring from 3 to 2 (observed
    # as every post-boundary MM1 waiting on the previous k's DVE add).
    warm_pool = ctx.enter_context(tc.tile_pool(name="warm", bufs=1))
    scratch = warm_pool.tile([128, 256], F16)
    nc.vector.memset(scratch[:], 0.0)
    pwarm = ptmp_pool.tile([128, 2 * Q], F32, name="pwarm", tag="pt")
    for _ in range(N_WARMUP):
        nc.tensor.matmul(pwarm[:, 0:256], lhsT=scratch[:, 0:128], rhs=scratch[:],
                         start=True, stop=True)

    def load_pair(bp, first=False):
        b0, b1 = 2 * bp, 2 * bp + 1
        q2 = q_pool.tile([128, DC, 2 * Q], F16, name=f"q2_{bp}", tag="q2")
        nc.sync.dma_start(q2[:].rearrange("p dc (h q) -> p dc h q", h=2),
                          qt[b0:b0 + 2].rearrange("h p dc q -> p dc h q"))
        if first:
            return (q2, b0, b1)
        da2 = da_pool.tile([128, ET, 2 * A], F16, name=f"da2_{bp}", tag="da2")
        nc.sync.dma_start(da2[:].rearrange("p et (h a) -> p et h a", h=2),
                          dat[b0:b0 + 2].rearrange("h p et a -> p et h a"))
        act_tiles[bp] = (q2, da2)

    def load_wk(k):
        wk = w_pool.tile([128, DC, E], F16, name=f"wk{k}", tag=f"wk{k}")
        nc.sync.dma_start(wk[:], w[k].rearrange("(dc p) e -> p dc e", p=128))
        return wk

    # Critical path: MM1(k0) needs q2_0 + wk0.  Put q2_0 (256KB) alone on the
    # scalar HWDGE queue and wk0 (128KB) first on the SP queue so both stream
    # concurrently the moment descriptors start flowing; the rest of SP's
    # queue follows consumer deadlines: MM1(k1)->MM2(k0)->DVE(k0)->ACT(k0).
    q2_0 = q_pool.tile([128, DC, 2 * Q], F16, name="q2_0", tag="q2")
    nc.scalar.dma_start(q2_0[:].rearrange("p dc (h q) -> p dc h q", h=2),
                        qt[0:2].rearrange("h p dc q -> p dc h q"))
    w_sb = {}
    w_sb[0] = load_wk(0)
    w_sb[1] = load_wk(1)
    da2_0 = da_pool.tile([128, ET, 2 * A], F16, name="da2_0", tag="da2")
    nc.sync.dma_start(da2_0[:].rearrange("p et (h a) -> p et h a", h=2),
                      dat[0:2].rearrange("h p et a -> p et h a"))
    act_tiles[0] = (q2_0, da2_0)
    # small constants ride on the (otherwise idle) scalar queue after q2_0
    vdt_sb = const_pool.tile([128, ET, 128], F32)
    nc.scalar.dma_start(vdt_sb[:], vdt.rearrange("et p k -> p et k"))
    mq_sb = const_pool.tile([128, QT, BL, K], F32)
    nc.scalar.dma_start(mq_sb[:], mq.rearrange("t p b k -> p t b k"))
    for k in range(2, K):
        w_sb[k] = load_wk(k)

    NBP = BL // 2
    # Completed collect tiles queue up here and drain ONE store per k on the
    # SP queue (a DGE config is ~0.8us of sequencer time; bursting 4 of them
    # at a chunk boundary stalled the pipeline in earlier traces).  Only the
    # end of the last batch-pair drains 2/k across both queues, plus a small
    # tapered final flush.
    pending = []  # (coll_tile, b, k_lo, cs, qt_i)
    nstore = [0]

    def issue_store(split_queues):
        coll_t, b, k_lo, cs, qt_i = pending.pop(0)
        eng = nc.scalar if (split_queues and nstore[0] % 2) else nc.sync
        nstore[0] += 1
        eng.dma_start(
            out[b, k_lo:k_lo + cs,
                qt_i * 128:(qt_i + 1) * 128, :].rearrange("k p a -> p k a"),
            coll_t[:],
        )

    for bp in range(NBP):
        b0, b1 = 2 * bp, 2 * bp + 1
        if bp not in act_tiles:
            load_pair(bp)
        if bp + 1 < NBP and bp + 1 not in act_tiles:
            load_pair(bp + 1)
        q2, da2 = act_tiles.pop(bp)

        # per-(b, qtile, k-chunk) collect tiles (fp16); chunks shrink toward
        # the end of the last batch-pair so the final store flush is small
        last_bp = bp == NBP - 1
        if not last_bp:
            chunk_sizes = [4, 4, 4, 4]
        else:
            chunk_sizes = [4, 4, 4, 2, 1, 1]
        k2chunk = {}
        koff = 0
        for ci, cs in enumerate(chunk_sizes):
            for off in range(cs):
                k2chunk[koff + off] = (ci, off, cs)
            koff += cs
        coll = {(h, qt_i, ci): out_pool.tile([128, cs, A], F16, name="coll", tag="coll")
                for h in (0, 1) for qt_i in range(QT)
                for ci, cs in enumerate(chunk_sizes)}

        for k in range(K):
            ptmps = []
            for et in range(ET):
                pt = ptmp_pool.tile([128, 2 * Q], F32, tag="pt")
                for dc in range(DC):
                    nc.tensor.matmul(
                        pt[:],
                        lhsT=w_sb[k][:, dc, et * 128:(et + 1) * 128],
                        rhs=q2[:, dc, :],
                        start=(dc == 0),
                        stop=(dc == DC - 1),
                    )
                ptmps.append(pt)
            tmp = tmp_pool.tile([128, ET, 2 * Q], F16)
            for et in range(ET):
                nc.vector.tensor_scalar_add(
                    tmp[:, et, :], ptmps[et][:], vdt_sb[:, et, k:k + 1]
                )
            for h, b in ((0, b0), (1, b1)):
                for qt_i in range(QT):
                    po = pout_pool.tile([128, A], F32)
                    for et in range(ET):
                        nc.tensor.matmul(
                            po[:],
                            lhsT=tmp[:, et, h * Q + qt_i * 128: h * Q + (qt_i + 1) * 128],
                            rhs=da2[:, et, h * A:(h + 1) * A],
                            start=(et == 0),
                            stop=(et == ET - 1),
                        )
                    nc.scalar.activation(
                        coll[(h, qt_i, k2chunk[k][0])][:, k2chunk[k][1], :], po[:], SIG,
                        bias=mq_sb[:, qt_i, b, k:k + 1],
                    )
            ci, off, cs = k2chunk[k]
            if off == cs - 1:
                k_lo = k - cs + 1
                for h, b in ((0, b0), (1, b1)):
                    for qt_i in range(QT):
                        pending.append((coll[(h, qt_i, ci)], b, k_lo, cs, qt_i))
            # endgame: drain 2/k, still SP-only (Scalar's sequencer must keep
            # feeding sigmoids; SP fits two ~0.8us configs in the 1.7us
            # k-period).  Both queues share only the post-loop flush.
            budget = 2 if (last_bp and k >= 8) else 1
            for _ in range(min(budget, len(pending))):
                issue_store(split_queues=False)

    while pending:
        issue_store(split_queues=True)


_COMPILED = None


def _get_compiled():
    global _COMPILED
    if _COMPILED is not None:
        return _COMPILED
    nc = bacc.Bacc("TRN2", target_bir_lowering=False, debug=False, num_devices=NCORES)
    aps = {
        "qt": nc.dram_tensor("qt", [BL, 128, D // 128, Q], F16, kind="ExternalInput").ap(),
        "dat": nc.dram_tensor("dat", [BL, 128, E // 128, A], F16, kind="ExternalInput").ap(),
        "w": nc.dram_tensor("w", [K, D, E], F16, kind="ExternalInput").ap(),
        "vdt": nc.dram_tensor("vdt", [E // 128, 128, 128], F32, kind="ExternalInput").ap(),
        "mq": nc.dram_tensor("mq", [Q // 128, 128, BL, K], F32, kind="ExternalInput").ap(),
        "out": nc.dram_tensor("out", [BL, K, Q, A], F16, kind="ExternalOutput").ap(),
    }
    with tile.TileContext(nc) as tc:
        with ExitStack() as ctx:
            _build_ntn(tc, ctx, aps)
    nc.compile()
    _COMPILED = nc
    return nc


def kernel(batch_q_em, batch_da_em, w, V, b):
    q = np.ascontiguousarray(np.asarray(batch_q_em, dtype=np.float32))
    da = np.ascontiguousarray(np.asarray(batch_da_em, dtype=np.float32))
    w = np.ascontiguousarray(np.asarray(w, dtype=np.float32))
    V = np.ascontiguousarray(np.asarray(V, dtype=np.float32))
    b = np.asarray(b, dtype=np.float32).reshape(-1)

    # packed to SBUF layout [b, p, dc, q] so each load is 128 x 1KB descriptors
    qt = np.ascontiguousarray(
        q.transpose(0, 2, 1).reshape(B, D // 128, 128, Q).transpose(0, 2, 1, 3)
    ).astype(np.float16)                                  # [B, 128, DC, Q]
    dat = np.ascontiguousarray(
        da.transpose(0, 2, 1).reshape(B, E // 128, 128, A).transpose(0, 2, 1, 3)
    ).astype(np.float16)                                  # [B, 128, ET, A]
    w16 = w.astype(np.float16)
    vdt_cols = np.ascontiguousarray(V[:, D:].T)          # [E, K]
    vdt = np.zeros((E // 128, 128, 128), dtype=np.float32)
    vdt[:, :, :K] = vdt_cols.reshape(E // 128, 128, K)
    # mq[b,q,k] = q[b] @ Vq^T + bias
    mqT = q @ V[:, :D].T + b[None, None, :]              # [B, Q, K]

    nc = _get_compiled()
    in_maps = []
    for c in range(NCORES):
        s = slice(c * BL, (c + 1) * BL)
        mq_shard = np.ascontiguousarray(
            mqT[s].reshape(BL, Q // 128, 128, K).transpose(1, 2, 0, 3)
        )  # [QT, 128, BL, K]
        in_maps.append({
            "qt": np.ascontiguousarray(qt[s]),
            "dat": np.ascontiguousarray(dat[s]),
            "w": w16,
            "vdt": vdt,
            "mq": mq_shard,
        })

    trace = bool(int(os.environ.get("NTN_TRACE", "0"))) and _install_profshim()
    res = bass_utils.run_bass_kernel_spmd(
        nc, in_maps, core_ids=list(range(NCORES)), trace=trace
    )
    if trace and res.exec_time_ns is not None:
        print(f"HW exec time: {res.exec_time_ns} ns")
    out = np.concatenate([r["out"] for r in res.results], axis=0).astype(np.float32)
    return out


# revision 18
# speedup vs baseline: 1.0891x; 1.0163x over previous
"""Trainium2 Bass kernel for the NTN problem.

out[b,k,q,a] = sigmoid( q[b,q,:] @ w[k] @ da[b,a,:]
                        + Vq[k]@q[b,q,:] + Vd[k]@da[b,a,:] + b[k] )

B=64, K=16, Q=A=D=256.  Sharding: data-parallel over batch B across the
8 NeuronCores (8 batches per core); w/V/b replicated.

Per core, per (k, batch-pair):
  MM1 (TensorE, fp16): tmp[e, q|q'] = sum_d w[k,d,e]^T qT[d, q|q']   (N=512)
  DVE: tmp PSUM->SBUF (fp16) with per-partition bias +Vd[k,e] (folds Vd@da)
  MM2 (TensorE, fp16): out[q, a] = sum_e tmp[e,q]^T daT[e, a]
  ScalarE: sigmoid(psum + bias mq[b,k,q]) where mq = Vq@q + b (host-prepped),
  written as fp16 into per-(b, qtile, k-chunk) collect tiles; the host
  upcasts the returned fp16 output to fp32 (quantization err ~2.4e-4,
  well inside the 2e-2 budget) which halves store traffic 32->16 MB/core.

Startup path (from NTFF traces): engine preambles run until ~5us, first
DMA descriptors flow ~8.7us.  The critical first loads are split across
the two HWDGE queues (scalar: q2 pair0; SP: w[0] first), with the rest of
the SP queue in consumer-deadline order (w1, da2, vdt, mq, w2..).  PE
warm-up matmuls on an (uninitialized) scratch tile run from the moment
the PE sequencer clears its preamble, so the HAM clock is at full speed
when the real stream starts; no memset gates them (the old GpSimd memset
pushed warm-up start to 7.7us).
"""

import os
import sys
import types
from contextlib import ExitStack

if "/opt/trn_rl_repo" not in sys.path:
    sys.path.insert(0, "/opt/trn_rl_repo")

import numpy as np

import concourse.bass as bass
import concourse.tile as tile
from concourse import bacc, bass_utils, mybir

F32 = mybir.dt.float32
F16 = mybir.dt.float16
SIG = mybir.ActivationFunctionType.Sigmoid

NCORES = 8
B, Q, A, D, K = 64, 256, 256, 256, 16
E = D
BL = B // NCORES

N_WARMUP = 15


def _install_profshim():
    """Provide antenv.axon_hooks so trace=True works under axon (best-effort)."""
    try:
        if "antenv.axon_hooks" in sys.modules:
            return True
        import antenv

        mod = types.ModuleType("antenv.axon_hooks")
        holder = {}
        mod.set_axon_ntff_profile_hook = lambda h: holder.__setitem__("h", h)
        mod.get_axon_ntff_profile_hook = lambda: holder.get("h")
        sys.modules["antenv.axon_hooks"] = mod
        antenv.axon_hooks = mod
        from trn_agent_boot.trn_boot import _ntff_profile_via_ctypes

        hook = _ntff_profile_via_ctypes("/opt/axon/libaxon_pjrt.so")
        if hook is None:
            return False
        mod.set_axon_ntff_profile_hook(hook)
        return True
    except Exception:
        return False


def _build_ntn(tc: tile.TileContext, ctx: ExitStack, aps: dict):
    nc = tc.nc
    DC, ET, QT = D // 128, E // 128, Q // 128
    qt, dat, w, vdt, mq, out = (aps[n] for n in ("qt", "dat", "w", "vdt", "mq", "out"))

    w_pool = ctx.enter_context(tc.tile_pool(name="w", bufs=1))
    const_pool = ctx.enter_context(tc.tile_pool(name="const", bufs=1))
    q_pool = ctx.enter_context(tc.tile_pool(name="q", bufs=3))
    da_pool = ctx.enter_context(tc.tile_pool(name="da", bufs=3))
    tmp_pool = ctx.enter_context(tc.tile_pool(name="tmp", bufs=4))
    out_pool = ctx.enter_context(tc.tile_pool(name="out", bufs=6))
    ptmp_pool = ctx.enter_context(tc.tile_pool(name="ptmp", bufs=3, space="PSUM"))
    pout_pool = ctx.enter_context(tc.tile_pool(name="pout", bufs=5, space="PSUM"))

    act_tiles = {}

    # PE warm-up: dummy matmuls on a scratch tile keep the PE busy from the
    # end of its sequencer preamble until the first real loads land, so HAM
    # reaches full clock before the real stream starts.  The memset rides on
    # DVE (idle until the first k's bias-add, and its preamble clears early;
    # the old GpSimd memset pushed warm-up start to 7.7us).  256-row warm
    # matmuls give fine granularity, so the warm-up end lands close to the
    # moment the first loads are ready.  pwarm shares ptmp's ring via the
    # explicit "pt" tag: a pool splits bufs across tags, so an untagged
    # pwarm would silently shrink the per-k pt ring from 3 to 2 (observed
    # as every post-boundary MM1 waiting on the previous k's DVE add).
    warm_pool = ctx.enter_context(tc.tile_pool(name="warm", bufs=1))
    scratch = warm_pool.tile([128, 256], F16)
    nc.vector.memset(scratch[:], 0.0)
    pwarm = ptmp_pool.tile([128, 2 * Q], F32, name="pwarm", tag="pt")
    for _ in range(N_WARMUP):
        nc.tensor.matmul(pwarm[:, 0:256], lhsT=scratch[:, 0:128], rhs=scratch[:],
                         start=True, stop=True)

    def load_pair(bp, first=False):
        b0, b1 = 2 * bp, 2 * bp + 1
        q2 = q_pool.tile([128, DC, 2 * Q], F16, name=f"q2_{bp}", tag="q2")
        nc.sync.dma_start(q2[:].rearrange("p dc (h q) -> p dc h q", h=2),
                          qt[b0:b0 + 2].rearrange("h p dc q -> p dc h q"))
        if first:
            return (q2, b0, b1)
        da2 = da_pool.tile([128, ET, 2 * A], F16, name=f"da2_{bp}", tag="da2")
        nc.sync.dma_start(da2[:].rearrange("p et (h a) -> p et h a", h=2),
                          dat[b0:b0 + 2].rearrange("h p et a -> p et h a"))
        act_tiles[bp] = (q2, da2)

    def load_wk(k):
        wk = w_pool.tile([128, DC, E], F16, name=f"wk{k}", tag=f"wk{k}")
        nc.sync.dma_start(wk[:], w[k].rearrange("(dc p) e -> p dc e", p=128))
        return wk

    # Critical path: MM1(k0) needs q2_0 + wk0.  Put q2_0 (256KB) alone on the
    # scalar HWDGE queue and wk0 (128KB) first on the SP queue so both stream
    # concurrently the moment descriptors start flowing; the rest of SP's
    # queue follows consumer deadlines: MM1(k1)->MM2(k0)->DVE(k0)->ACT(k0).
    q2_0 = q_pool.tile([128, DC, 2 * Q], F16, name="q2_0", tag="q2")
    nc.scalar.dma_start(q2_0[:].rearrange("p dc (h q) -> p dc h q", h=2),
                        qt[0:2].rearrange("h p dc q -> p dc h q"))
    w_sb = {}
    w_sb[0] = load_wk(0)
    da2_0 = da_pool.tile([128, ET, 2 * A], F16, name="da2_0", tag="da2")
    nc.sync.dma_start(da2_0[:].rearrange("p et (h a) -> p et h a", h=2),
                      dat[0:2].rearrange("h p et a -> p et h a"))
    act_tiles[0] = (q2_0, da2_0)
    w_sb[1] = load_wk(1)
    # vdt (DVE k0's bias) rides the scalar queue behind q2_0; mq (first
    # sigmoid's bias) follows w1 on SP — both land just ahead of their
    # first consumers.
    vdt_sb = const_pool.tile([128, ET, 128], F32)
    nc.scalar.dma_start(vdt_sb[:], vdt.rearrange("et p k -> p et k"))
    mq_sb = const_pool.tile([128, QT, BL, K], F32)
    nc.sync.dma_start(mq_sb[:], mq.rearrange("t p b k -> p t b k"))
    for k in range(2, K):
        w_sb[k] = load_wk(k)

    NBP = BL // 2
    # One collect tile per k-chunk covers both batches and both q-tiles
    # ([128, h, qt, cs, A]), so a chunk ships as a SINGLE dma_start (fewer
    # ~0.8-1.3us DGE configs serializing on the sequencers, and a shorter
    # end-of-kernel queue drain).  Chunks complete at their boundary k and
    # the store issues right there on the SP queue; the last two tapered
    # chunks split h across SP/Scalar so the final flush runs two configs
    # in parallel.
    for bp in range(NBP):
        b0, b1 = 2 * bp, 2 * bp + 1
        if bp not in act_tiles:
            load_pair(bp)
        if bp + 1 < NBP and bp + 1 not in act_tiles:
            load_pair(bp + 1)
        q2, da2 = act_tiles.pop(bp)

        last_bp = bp == NBP - 1
        if not last_bp:
            chunk_sizes = [4, 4, 4, 4]
        else:
            chunk_sizes = [4, 4, 4, 2, 1, 1]
        k2chunk = {}
        koff = 0
        for ci, cs in enumerate(chunk_sizes):
            for off in range(cs):
                k2chunk[koff + off] = (ci, off, cs)
            koff += cs
        coll = {ci: out_pool.tile([128, 2, QT, cs, A], F16, name="coll", tag="coll")
                for ci, cs in enumerate(chunk_sizes)}

        for k in range(K):
            ptmps = []
            for et in range(ET):
                pt = ptmp_pool.tile([128, 2 * Q], F32, tag="pt")
                for dc in range(DC):
                    nc.tensor.matmul(
                        pt[:],
                        lhsT=w_sb[k][:, dc, et * 128:(et + 1) * 128],
                        rhs=q2[:, dc, :],
                        start=(dc == 0),
                        stop=(dc == DC - 1),
                    )
                ptmps.append(pt)
            tmp = tmp_pool.tile([128, ET, 2 * Q], F16)
            for et in range(ET):
                nc.vector.tensor_scalar_add(
                    tmp[:, et, :], ptmps[et][:], vdt_sb[:, et, k:k + 1]
                )
            for h, b in ((0, b0), (1, b1)):
                for qt_i in range(QT):
                    po = pout_pool.tile([128, A], F32)
                    for et in range(ET):
                        nc.tensor.matmul(
                            po[:],
                            lhsT=tmp[:, et, h * Q + qt_i * 128: h * Q + (qt_i + 1) * 128],
                            rhs=da2[:, et, h * A:(h + 1) * A],
                            start=(et == 0),
                            stop=(et == ET - 1),
                        )
                    nc.scalar.activation(
                        coll[k2chunk[k][0]][:, h, qt_i, k2chunk[k][1], :], po[:], SIG,
                        bias=mq_sb[:, qt_i, b, k:k + 1],
                    )
            ci, off, cs = k2chunk[k]
            if off == cs - 1:
                k_lo = k - cs + 1
                # out DRAM layout is [b, q, k, a] so (k, a) merges into one
                # contiguous 512*cs-byte run per (b, q) — 3 free dims (DMA AP
                # limit) and fat descriptors; host restores [b, k, q, a].
                dram = out[b0:b0 + 2, :, k_lo:k_lo + cs].rearrange(
                    "h (qt p) k a -> p h qt k a", p=128)
                if last_bp and ci >= len(chunk_sizes) - 2:
                    # final taper chunks: h=0 on SP, h=1 on Scalar, in parallel
                    nc.sync.dma_start(dram[:, 0], coll[ci][:, 0])
                    nc.scalar.dma_start(dram[:, 1], coll[ci][:, 1])
                else:
                    nc.sync.dma_start(dram, coll[ci][:])


_COMPILED = None


def _get_compiled():
    global _COMPILED
    if _COMPILED is not None:
        return _COMPILED
    nc = bacc.Bacc("TRN2", target_bir_lowering=False, debug=False, num_devices=NCORES)
    aps = {
        "qt": nc.dram_tensor("qt", [BL, 128, D // 128, Q], F16, kind="ExternalInput").ap(),
        "dat": nc.dram_tensor("dat", [BL, 128, E // 128, A], F16, kind="ExternalInput").ap(),
        "w": nc.dram_tensor("w", [K, D, E], F16, kind="ExternalInput").ap(),
        "vdt": nc.dram_tensor("vdt", [E // 128, 128, 128], F32, kind="ExternalInput").ap(),
        "mq": nc.dram_tensor("mq", [Q // 128, 128, BL, K], F32, kind="ExternalInput").ap(),
        "out": nc.dram_tensor("out", [BL, Q, K, A], F16, kind="ExternalOutput").ap(),
    }
    with tile.TileContext(nc) as tc:
        with ExitStack() as ctx:
            _build_ntn(tc, ctx, aps)
    nc.compile()
    _COMPILED = nc
    return nc


def kernel(batch_q_em, batch_da_em, w, V, b):
    q = np.ascontiguousarray(np.asarray(batch_q_em, dtype=np.float32))
    da = np.ascontiguousarray(np.asarray(batch_da_em, dtype=np.float32))
    w = np.ascontiguousarray(np.asarray(w, dtype=np.float32))
    V = np.ascontiguousarray(np.asarray(V, dtype=np.float32))
    b = np.asarray(b, dtype=np.float32).reshape(-1)

    # packed to SBUF layout [b, p, dc, q] so each load is 128 x 1KB descriptors
    qt = np.ascontiguousarray(
        q.transpose(0, 2, 1).reshape(B, D // 128, 128, Q).transpose(0, 2, 1, 3)
    ).astype(np.float16)                                  # [B, 128, DC, Q]
    dat = np.ascontiguousarray(
        da.transpose(0, 2, 1).reshape(B, E // 128, 128, A).transpose(0, 2, 1, 3)
    ).astype(np.float16)                                  # [B, 128, ET, A]
    w16 = w.astype(np.float16)
    vdt_cols = np.ascontiguousarray(V[:, D:].T)          # [E, K]
    vdt = np.zeros((E // 128, 128, 128), dtype=np.float32)
    vdt[:, :, :K] = vdt_cols.reshape(E // 128, 128, K)
    # mq[b,q,k] = q[b] @ Vq^T + bias
    mqT = q @ V[:, :D].T + b[None, None, :]              # [B, Q, K]

    nc = _get_compiled()
    in_maps = []
    for c in range(NCORES):
        s = slice(c * BL, (c + 1) * BL)
        mq_shard = np.ascontiguousarray(
            mqT[s].reshape(BL, Q // 128, 128, K).transpose(1, 2, 0, 3)
        )  # [QT, 128, BL, K]
        in_maps.append({
            "qt": np.ascontiguousarray(qt[s]),
            "dat": np.ascontiguousarray(dat[s]),
            "w": w16,
            "vdt": vdt,
            "mq": mq_shard,
        })

    trace = bool(int(os.environ.get("NTN_TRACE", "0"))) and _install_profshim()
    res = bass_utils.run_bass_kernel_spmd(
        nc, in_maps, core_ids=list(range(NCORES)), trace=trace
    )
    if trace and res.exec_time_ns is not None:
        print(f"HW exec time: {res.exec_time_ns} ns")
    out = np.concatenate([r["out"] for r in res.results], axis=0)  # [B, Q, K, A] f16
    return np.ascontiguousarray(out.transpose(0, 2, 1, 3), dtype=np.float32)
